# revision 16
# baseline (speedup 1.0000x reference)
"""Trainium2 Bass kernel for nn_Meta_74010876445283 (topk_masking).

Computes, from x [256,3,300,25,2], x_T [153600,256], labels [153600]:
  - per-class stats -> loss = Sw / (Sb + eps)
  - frame scores -> top-64 frames per sample -> gather x -> x_select

Distribution: data-parallel over N (samples) across 8 NeuronCores; one tiny
AllReduce of per-class sums/counts/sumsq; everything else core-local.

Math layout (per core, 32 samples, 19200 rows of x_T):
  pass 1 (over 150 row-tiles of 128):
    onehot[128,6] = [label==k for k in 0..4 | ones]
    psum_stats[6,257] += onehot.T @ [X | 1]   (rows 0-4: class sums+counts,
                                               row 5: overall sum + total)
    rowsumsq[:,t] = sum(X^2) per row           (ACT Square + accumulate)
  AllReduce(stats, local sumsq total)
  pass 2: mu = sums/counts; M_r = onehot @ mu (bf16 matmul);
    dot_i = sum(x_i * M_i)  (DVE scalar_tensor_tensor with accum)
    intra = rowsumsq - 2*dot + msq[label] + eps
    score = 0.5*inter[label] / intra;  VF[n,t] = score(m=0)+score(m=1)
  topk via DVE max/match_replace (8 rounds of top-8, twice: once to find the
    top-64 set, once over mask*(1024-t) to emit indices in ascending order)
  gather x rows via gpsimd.dma_gather from a 256B-padded copy of x.
"""

import sys

sys.path.insert(0, "/opt/trn_rl_repo")

import os
import numpy as np
import ml_dtypes

STAGE = int(os.environ.get("KERNEL_STAGE", "3"))

from concourse import bass, mybir, tile, bacc, bass_utils, library_config

F32 = mybir.dt.float32
BF16 = mybir.dt.bfloat16
I16 = mybir.dt.int16

P = 128
NCORES = 8
N_CLASSES = 5
EPS = 1e-8
NEG = -1.0e30

# problem constants
N, M, T, C, V, TK, D = 256, 2, 300, 3, 25, 64, 256
NL = N // NCORES                  # 32 samples per core
RL = NL * M * T                   # 19200 rows per core
NT = RL // P                      # 150 tiles
SLOT = D + 2                      # X slot stride (256 data + ones col + pad)
GROWS = NL * C * T                # 28800 gather-source rows per core
GIDX = NL * C * TK                # 6144 gathered rows per core
GBLK = GIDX // P                  # 48
OHT_CHUNK = 10                    # pass-2 onehotT tiles per DMA chunk
DMA_GRP = 10                      # x_T tiles per DMA


def _build_kernel(tc):
    nc = tc.nc
    AL = mybir.AluOpType
    AF = mybir.ActivationFunctionType
    AX = mybir.AxisListType

    xt = nc.dram_tensor("xt", [RL, D], F32, kind="ExternalInput").ap()
    lab = nc.dram_tensor("lab", [P, NT], F32, kind="ExternalInput").ap()
    ohT = nc.dram_tensor("ohT", [N_CLASSES, RL], BF16, kind="ExternalInput").ap()
    xg = nc.dram_tensor("xg", [GROWS, 64], F32, kind="ExternalInput").ap()
    krow = nc.dram_tensor("krow", [P, N_CLASSES], F32, kind="ExternalInput").ap()
    i5 = nc.dram_tensor("i5", [N_CLASSES, N_CLASSES], F32, kind="ExternalInput").ap()
    w0 = nc.dram_tensor("w0", [NL, T], F32, kind="ExternalInput").ap()
    c3 = nc.dram_tensor("c3", [NL, C], F32, kind="ExternalInput").ap()
    loss_o = nc.dram_tensor("loss_o", [1, 1], F32, kind="ExternalOutput").ap()
    xsel_o = nc.dram_tensor("xsel_o", [GIDX, C * V * M // C], F32,
                            kind="ExternalOutput").ap()  # [6144, 50]

    ctx = _CACHED["exitstack"]
    res = ctx.enter_context(tc.tile_pool(name="res", bufs=1))
    work = ctx.enter_context(tc.tile_pool(name="work", bufs=2))
    psum = ctx.enter_context(tc.tile_pool(name="psum", bufs=2, space="PSUM"))
    psum1 = ctx.enter_context(tc.tile_pool(name="psum1", bufs=1, space="PSUM"))
    dram = ctx.enter_context(tc.tile_pool(name="dram", bufs=1, space="DRAM"))

    # ---------------- resident tensors ----------------
    X = res.tile([P, NT * SLOT], F32, tag="X")          # 150 slots of [*,258]
    Xs = X.rearrange("p (t s) -> p t s", s=SLOT)
    labs = res.tile([P, NT], F32, tag="labs")
    rowsumsq = res.tile([P, NT], F32, tag="rsq")
    dots = res.tile([P, NT], F32, tag="dots")
    krow_s = res.tile([P, N_CLASSES], F32, tag="krow")
    i5_s = res.tile([N_CLASSES, N_CLASSES], F32, tag="i5")
    w0_s = res.tile([NL, T], F32, tag="w0")
    c3_s = res.tile([NL, C], F32, tag="c3")
    ones128 = res.tile([P, 1], F32, tag="ones128")
    mu = res.tile([N_CLASSES, D], F32, tag="mu")
    mu_bf = res.tile([N_CLASSES, D], BF16, tag="mubf")
    gstats = res.tile([N_CLASSES, D + 2], F32, tag="gstats")
    interh_b = res.tile([P, N_CLASSES], F32, tag="interhb")
    msq_b = res.tile([P, N_CLASSES], F32, tag="msqb")
    msq_sel = res.tile([P, NT], F32, tag="msqsel")
    inter_sel = res.tile([P, NT], F32, tag="intersel")

    # dma_gather + partition_broadcast live in the "mlp" GPSIMD library
    nc.gpsimd.load_library(library_config.mlp)

    nc.sync.dma_start(labs[:, :], lab)
    nc.sync.dma_start(krow_s[:, :], krow)
    nc.sync.dma_start(i5_s[:, :], i5)
    nc.sync.dma_start(w0_s[:, :], w0)
    nc.sync.dma_start(c3_s[:, :], c3)
    nc.vector.memset(ones128[:, :], 1.0)
    # ones column of every X slot
    nc.vector.memset(Xs[:, :, D:D + 1], 1.0)

    # ---------------- pass 1 ----------------
    psum_stats = psum1.tile([N_CLASSES, D + 1], F32, tag="stats")
    xt_t = xt.rearrange("(t p) d -> p t d", p=P)        # [128, 150, 256]
    for g in range(NT // DMA_GRP):
        nc.sync.dma_start(
            Xs[:, g * DMA_GRP:(g + 1) * DMA_GRP, 0:D],
            xt_t[:, g * DMA_GRP:(g + 1) * DMA_GRP, :],
        )

    for t in range(NT):
        oh = work.tile([P, N_CLASSES], F32, tag="oh")
        nc.vector.tensor_scalar(
            out=oh[:, :], in0=krow_s[:, :], scalar1=labs[:, t:t + 1],
            scalar2=None, op0=AL.is_equal,
        )
        nc.tensor.matmul(
            psum_stats[:, :], oh[:, :], Xs[:, t, 0:D + 1],
            start=(t == 0), stop=(t == NT - 1),
        )
        scr = work.tile([P, D], F32, tag="scratch")
        nc.scalar.activation(
            out=scr[:, :], in_=Xs[:, t, 0:D], func=AF.Square,
            accum_out=rowsumsq[:, t:t + 1],
        )

    # local total sumsq -> scalar
    rs1 = res.tile([P, 1], F32, tag="rs1")
    nc.vector.tensor_reduce(rs1[:, :], rowsumsq[:, :], axis=AX.X, op=AL.add)
    psum_s1 = psum1.tile([1, 1], F32, tag="small", bufs=2)
    nc.tensor.matmul(psum_s1[:, :], rs1[:, :], ones128[:, :], start=True, stop=True)

    payload = res.tile([N_CLASSES, D + 2], F32, tag="payload")
    nc.vector.tensor_copy(payload[:, 0:D + 1], psum_stats[:, :])
    nc.vector.memset(payload[:, D + 1:D + 2], 0.0)
    nc.vector.tensor_copy(payload[0:1, D + 1:D + 2], psum_s1[:, :])

    cc_in = dram.tile([N_CLASSES, D + 2], F32)
    cc_out = dram.tile([N_CLASSES, D + 2], F32, addr_space="Shared")
    nc.gpsimd.dma_start(cc_in[:, :], payload[:, :])
    nc.gpsimd.collective_compute(
        "AllReduce", AL.add,
        replica_groups=[list(range(NCORES))],
        ins=[cc_in.opt()],
        outs=[cc_out.opt()],
    )
    nc.gpsimd.dma_start(gstats[:, :], cc_out[:, :])

    # ---------------- post-AllReduce stats (tiny) ----------------
    cnt5 = res.tile([N_CLASSES, 1], F32, tag="cnt5")
    rc5 = res.tile([N_CLASSES, 1], F32, tag="rc5")
    nc.vector.tensor_scalar(out=cnt5[:, :], in0=gstats[:, D:D + 1], scalar1=1.0,
                            scalar2=None, op0=AL.max)
    nc.vector.reciprocal(rc5[:, :], cnt5[:, :])
    nc.vector.tensor_scalar(out=mu[:, :], in0=gstats[:, 0:D], scalar1=rc5[:, :],
                            scalar2=None, op0=AL.mult)
    nc.vector.tensor_copy(mu_bf[:, :], mu[:, :])
    # overall mean: sum class sums (+ total count), scale, broadcast to 5 rows
    ones5 = res.tile([N_CLASSES, 1], F32, tag="ones5")
    nc.vector.memset(ones5[:, :], 1.0)
    psum_os = psum1.tile([1, D + 1], F32, tag="small", bufs=2)
    nc.tensor.matmul(psum_os[:, :], ones5[:, :], gstats[:, 0:D + 1], start=True,
                     stop=True)
    rct = res.tile([1, 1], F32, tag="rct")
    nc.vector.reciprocal(rct[:, :], psum_os[0:1, D:D + 1])
    muo = res.tile([1, D], F32, tag="muo")
    nc.vector.tensor_scalar(out=muo[:, :], in0=psum_os[0:1, 0:D],
                            scalar1=rct[:, :], scalar2=None, op0=AL.mult)
    ones15 = res.tile([1, N_CLASSES], F32, tag="ones15")
    nc.vector.memset(ones15[:, :], 1.0)
    psum_mo5 = psum1.tile([N_CLASSES, D], F32, tag="small", bufs=2)
    nc.tensor.matmul(psum_mo5[:, :], ones15[:, :], muo[:, :], start=True,
                     stop=True)
    muo5 = res.tile([N_CLASSES, D], F32, tag="muo5")
    nc.vector.tensor_copy(muo5[:, :], psum_mo5[:, :])
    dmu = res.tile([N_CLASSES, D], F32, tag="dmu")
    nc.vector.tensor_tensor(dmu[:, :], mu[:, :], muo5[:, :], op=AL.subtract)
    scr5 = res.tile([N_CLASSES, D], F32, tag="scr5")
    inter5 = res.tile([N_CLASSES, 1], F32, tag="inter5")
    msq5 = res.tile([N_CLASSES, 1], F32, tag="msq5")
    nc.vector.scalar_tensor_tensor(
        out=scr5[:, :], in0=dmu[:, :], scalar=1.0, in1=dmu[:, :],
        op0=AL.mult, op1=AL.mult, accum_out=inter5[:, :])
    nc.vector.scalar_tensor_tensor(
        out=scr5[:, :], in0=mu[:, :], scalar=1.0, in1=mu[:, :],
        op0=AL.mult, op1=AL.mult, accum_out=msq5[:, :])
    # Sb = sum(counts*inter), Swc = sum(counts*msq)
    sd = res.tile([N_CLASSES, 2], F32, tag="sd")
    nc.vector.tensor_tensor(sd[:, 0:1], gstats[0:N_CLASSES, D:D + 1], inter5[:, :],
                            op=AL.mult)
    nc.vector.tensor_tensor(sd[:, 1:2], gstats[0:N_CLASSES, D:D + 1], msq5[:, :],
                            op=AL.mult)
    psum_red = psum1.tile([1, 2], F32, tag="small", bufs=2)
    nc.tensor.matmul(psum_red[:, :], ones5[:, :], sd[:, :], start=True, stop=True)
    sw = res.tile([1, 1], F32, tag="sw")
    nc.vector.tensor_tensor(sw[:, :], gstats[0:1, D + 1:D + 2], psum_red[0:1, 1:2],
                            op=AL.subtract)
    sbe = res.tile([1, 1], F32, tag="sbe")
    nc.vector.tensor_scalar(out=sbe[:, :], in0=psum_red[0:1, 0:1], scalar1=EPS,
                            scalar2=None, op0=AL.add)
    rb = res.tile([1, 1], F32, tag="rb")
    nc.vector.reciprocal(rb[:, :], sbe[:, :])
    loss_sb = res.tile([1, 1], F32, tag="loss")
    nc.vector.tensor_tensor(loss_sb[:, :], sw[:, :], rb[:, :], op=AL.mult)
    nc.sync.dma_start(loss_o, loss_sb[:, :])

    # broadcast 0.5*inter and msq to all partitions as rows
    interh5 = res.tile([N_CLASSES, 1], F32, tag="interh5")
    nc.vector.tensor_scalar(out=interh5[:, :], in0=inter5[:, :], scalar1=0.5,
                            scalar2=None, op0=AL.mult)
    psum_tra = psum1.tile([1, N_CLASSES], F32, tag="small", bufs=2)
    psum_trb = psum1.tile([1, N_CLASSES], F32, tag="small", bufs=2)
    nc.tensor.matmul(psum_tra[:, :], interh5[:, :], i5_s[:, :], start=True,
                     stop=True)
    nc.tensor.matmul(psum_trb[:, :], msq5[:, :], i5_s[:, :], start=True, stop=True)
    trs_a = res.tile([1, N_CLASSES], F32, tag="trsa")
    trs_b = res.tile([1, N_CLASSES], F32, tag="trsb")
    nc.vector.tensor_copy(trs_a[:, :], psum_tra[:, :])
    nc.vector.tensor_copy(trs_b[:, :], psum_trb[:, :])
    nc.gpsimd.partition_broadcast(interh_b[:, :], trs_a[:, :])
    nc.gpsimd.partition_broadcast(msq_b[:, :], trs_b[:, :])

    # msq_sel / inter_sel : per-row msq[label], 0.5*inter[label]
    for k in range(N_CLASSES):
        tm = work.tile([P, NT], F32, tag="seltmp")
        nc.vector.tensor_scalar(
            out=tm[:, :], in0=labs[:, :], scalar1=float(k),
            scalar2=msq_b[:, k:k + 1], op0=AL.is_equal, op1=AL.mult)
        if k == 0:
            nc.vector.tensor_scalar(out=msq_sel[:, :], in0=tm[:, :], scalar1=EPS,
                                    scalar2=None, op0=AL.add)
        else:
            nc.vector.tensor_tensor(msq_sel[:, :], msq_sel[:, :], tm[:, :],
                                    op=AL.add)
        tm2 = work.tile([P, NT], F32, tag="seltmp")
        nc.vector.tensor_scalar(
            out=tm2[:, :], in0=labs[:, :], scalar1=float(k),
            scalar2=interh_b[:, k:k + 1], op0=AL.is_equal, op1=AL.mult)
        if k == 0:
            nc.vector.tensor_copy(inter_sel[:, :], tm2[:, :])
        else:
            nc.vector.tensor_tensor(inter_sel[:, :], inter_sel[:, :], tm2[:, :],
                                    op=AL.add)

    # ---------------- pass 2: dots ----------------
    ohT_t = ohT  # [5, 19200]
    n_chunks = NT // OHT_CHUNK
    for ch in range(n_chunks):
        ohc = work.tile([N_CLASSES, OHT_CHUNK * P], BF16, tag="ohc")
        nc.sync.dma_start(
            ohc[:, :], ohT_t[:, ch * OHT_CHUNK * P:(ch + 1) * OHT_CHUNK * P])
        for tt in range(OHT_CHUNK):
            t = ch * OHT_CHUNK + tt
            pm = psum.tile([P, D], F32, tag="mr")
            nc.tensor.matmul(pm[:, :], ohc[:, tt * P:(tt + 1) * P], mu_bf[:, :],
                             start=True, stop=True)
            scr2 = work.tile([P, D], BF16, tag="scr2")
            nc.vector.scalar_tensor_tensor(
                out=scr2[:, :], in0=Xs[:, t, 0:D], scalar=1.0, in1=pm[:, :],
                op0=AL.mult, op1=AL.mult, accum_out=dots[:, t:t + 1])

    # intra = rowsumsq - 2*dots + msq_sel(+eps); scores = inter_sel / intra
    intra = res.tile([P, NT], F32, tag="intra")
    nc.vector.scalar_tensor_tensor(
        out=intra[:, :], in0=dots[:, :], scalar=-2.0, in1=rowsumsq[:, :],
        op0=AL.mult, op1=AL.add)
    nc.vector.tensor_tensor(intra[:, :], intra[:, :], msq_sel[:, :], op=AL.add)
    rint = res.tile([P, NT], F32, tag="rint")
    nc.vector.reciprocal(rint[:, :], intra[:, :])
    scores = res.tile([P, NT], F32, tag="scores")
    nc.vector.tensor_tensor(scores[:, :], rint[:, :], inter_sel[:, :], op=AL.mult)

    if STAGE < 1:
        return
    # ---------------- reorder scores -> [NL, M*T] via DRAM ----------------
    sc_d = dram.tile([RL], F32)
    nc.sync.dma_start(sc_d.rearrange("(t p) -> p t", p=P), scores[:, :])
    scN = res.tile([NL, M * T], F32, tag="scN")
    nc.sync.dma_start(scN[:, :], sc_d.rearrange("(n j) -> n j", j=M * T))
    VF = res.tile([NL, T], F32, tag="VF")
    nc.vector.tensor_tensor(VF[:, :], scN[:, 0:T], scN[:, T:2 * T], op=AL.add)

    # ---------------- top-64 ----------------
    vfa = work.tile([NL, T], F32, tag="vfa")
    vfb = work.tile([NL, T], F32, tag="vfb")
    nc.vector.tensor_copy(vfa[:, :], VF[:, :])
    cur, nxt = vfa, vfb
    for r in range(TK // 8):
        m8 = work.tile([NL, 8], F32, tag="m8")
        nc.vector.max(m8[:, :], cur[:, :])
        nc.vector.match_replace(nxt[:, :], m8[:, :], cur[:, :], NEG)
        cur, nxt = nxt, cur
    mask = work.tile([NL, T], F32, tag="mask")
    nc.vector.tensor_tensor(mask[:, :], cur[:, :], VF[:, :], op=AL.not_equal)
    wv = work.tile([NL, T], F32, tag="wv")
    nc.vector.tensor_tensor(wv[:, :], mask[:, :], w0_s[:, :], op=AL.mult)
    wv2 = work.tile([NL, T], F32, tag="wv2")
    idxv = res.tile([NL, TK], F32, tag="idxv")
    cur, nxt = wv, wv2
    for r in range(TK // 8):
        nc.vector.max(idxv[:, r * 8:(r + 1) * 8], cur[:, :])
        nc.vector.match_replace(nxt[:, :], idxv[:, r * 8:(r + 1) * 8], cur[:, :],
                                0.0)
        cur, nxt = nxt, cur
    # t = 1024 - w ; global row = c3[n,c] + t
    tvals = res.tile([NL, TK], F32, tag="tvals")
    nc.vector.tensor_scalar(out=tvals[:, :], in0=idxv[:, :], scalar1=-1.0,
                            scalar2=1024.0, op0=AL.mult, op1=AL.add)
    val32 = res.tile([NL, C * TK], F32, tag="val32")
    for c in range(C):
        nc.vector.tensor_scalar(
            out=val32[:, c * TK:(c + 1) * TK], in0=tvals[:, :],
            scalar1=c3_s[:, c:c + 1], scalar2=None, op0=AL.add)
    idx16 = res.tile([NL, C * TK], I16, tag="idx16")
    nc.vector.tensor_copy(idx16[:, :], val32[:, :])

    if STAGE < 2:
        return
    idx_d = dram.tile([GIDX], I16)
    nc.sync.dma_start(idx_d.rearrange("(n j) -> n j", j=C * TK), idx16[:, :])
    # dma_gather wants [128, GIDX//16]: idx i at partition i%16, replicated
    # to all 8 groups of 16 partitions (one per Q7 core)
    idxs_sb = res.tile([P, GIDX // 16], I16, tag="idxs")
    for g in range(8):
        nc.sync.dma_start(idxs_sb[16 * g:16 * (g + 1), :],
                          idx_d.rearrange("(f p) -> p f", p=16))

    if STAGE < 3:
        return
    # ---------------- gather + write out ----------------
    gat = res.tile([P, GBLK * 64], F32, tag="gat")
    nc.gpsimd.dma_gather(
        out_ap=gat.rearrange("p (g e) -> p g e", e=64),
        in_ap=xg,
        idxs_ap=idxs_sb[:, :],
        num_idxs=GIDX,
        num_idxs_reg=GIDX,
        elem_size=64,
        elem_step=64,
        single_packet=False,
    )
    gat3 = gat.rearrange("p (g e) -> p g e", e=64)
    nc.sync.dma_start(
        xsel_o.rearrange("(g p) e -> p g e", p=P),
        gat3[:, :, 0:50],
    )


_CACHED = {}


def _get_nc():
    if "nc" not in _CACHED:
        import contextlib
        nc = bacc.Bacc("TRN2", target_bir_lowering=False, debug=False,
                       num_devices=NCORES)
        with tile.TileContext(nc, trace_sim=False) as tc:
            with contextlib.ExitStack() as stack:
                _CACHED["exitstack"] = stack
                _build_kernel(tc)
        nc.compile()
        _CACHED["nc"] = nc
    return _CACHED["nc"]


def _host_prep(x, x_T, labels):
    """Build per-core input maps."""
    x = np.ascontiguousarray(np.asarray(x, dtype=np.float32))
    x_T = np.ascontiguousarray(np.asarray(x_T, dtype=np.float32))
    labels = np.asarray(labels).astype(np.int32)

    krow = np.broadcast_to(np.arange(N_CLASSES, dtype=np.float32), (P, N_CLASSES))
    krow = np.ascontiguousarray(krow)
    i5 = np.eye(N_CLASSES, dtype=np.float32)
    w0 = np.ascontiguousarray(
        np.broadcast_to(1024.0 - np.arange(T, dtype=np.float32), (NL, T)))
    c3 = (300.0 * (3.0 * np.arange(NL, dtype=np.float32)[:, None]
                   + np.arange(C, dtype=np.float32)[None, :]))
    c3 = np.ascontiguousarray(c3.astype(np.float32))

    in_maps = []
    for r in range(NCORES):
        lo = r * RL
        xt_r = x_T[lo:lo + RL]
        lab_r = labels[lo:lo + RL]
        lab_pt = np.ascontiguousarray(
            lab_r.astype(np.float32).reshape(NT, P).T)
        ohT = (lab_r[None, :] == np.arange(N_CLASSES)[:, None])
        ohT = np.ascontiguousarray(ohT.astype(ml_dtypes.bfloat16))
        # gather source: rows (n,c,t) padded to 64 floats
        xr = x[r * NL:(r + 1) * NL]                     # [32, 3, 300, 25, 2]
        xg = np.zeros((GROWS, 64), dtype=np.float32)
        xg[:, 0:50] = xr.reshape(GROWS, 50)
        in_maps.append({
            "xt": xt_r,
            "lab": lab_pt,
            "ohT": ohT,
            "xg": xg,
            "krow": krow,
            "i5": i5,
            "w0": w0,
            "c3": c3,
        })
    return in_maps


def _assemble(results):
    loss = np.float32(results[0]["loss_o"].reshape(-1)[0])
    xs = np.stack([results[r]["xsel_o"].reshape(NL, C, TK, V, M)
                   for r in range(NCORES)])
    x_select = np.ascontiguousarray(xs.reshape(N, C, TK, V, M))
    return loss, x_select


def run(x, x_T, labels, trace=False):
    nc = _get_nc()
    in_maps = _host_prep(x, x_T, labels)
    res = bass_utils.run_bass_kernel_spmd(
        nc, in_maps, core_ids=list(range(NCORES)), trace=trace)
    loss, x_select = _assemble(res.results)
    return loss, x_select, res


def kernel(x, x_T, labels, N=None, M=None, T=None, topk=None):
    loss, x_select, _ = run(x, x_T, labels, trace=False)
    return loss, x_select
